# revision 15
# baseline (speedup 1.0000x reference)
r"""Causal multi-head attention (B=4, T=2048, C=1024, H=16, D=64) on 8 TRN2 NeuronCores.

Sharding: core = (batch b, head-group hg).  b = core // 2, hg = core % 2.
Each core computes, for its batch, the attention-output contribution of its
8 heads, including the qkv projection restricted to those heads' columns and
the o-projection restricted to those heads' rows.  The two cores sharing a
batch produce partial sums of the o-projection; the host adds them together
with the (analytically folded) v-bias/o-bias correction.

Math notes:
  - k-bias contributes only q-row-constant score shifts, which cancel in
    softmax, so it is dropped; only the q bias is applied on device.
  - v bias and o bias are affine post-softmax:  (P@(V + 1 b_v^T))@Wo + b_o =
    (P@V)@Wo + (b_v@Wo + b_o), folded into a host-side correction row.
  - Softmax is computed without max subtraction (scores are O(1) here), as
    exp(s/8) accumulated straight into the PV matmul; the denominator rides
    as a 65th output row via a ones column interleaved into the V layout.

On-device dataflow per core (matmuls in float32r: full-speed fp32 path):
  xT [C=1024, T=2048] (host pre-transposed) -> SBUF
  QK^T = Wqk^T @ xT  -> [1024, 2048]  (q rows 0-511 with bias, k rows 512-1023)
  V    = xT^T @ Wv   -> [2048, 8, 65] (natural layout + ones column)
  per head h, per 512-wide q chunk:
     S^T tile [tk=128, tq<=512] = K_h @ Q_h^T   (causal-skipped tiles omitted)
     E = exp(S^T / 8)   (diagonal tiles masked with a triangular 0/1 mask)
     [O^T; denom] += [V_h | 1]^T @ E    (one PSUM accumulation group, M=65)
     O^T_norm = O^T * broadcast(1/denom)
  out_partial [2048, 1024] = concat_h(O^T_norm)^T @ Wo_shard  -> DRAM
"""

import sys

sys.path.insert(0, "/opt/trn_rl_repo")

import numpy as np

import concourse.bass as bass
import concourse.tile as tile
from concourse import bacc, mybir
from concourse.bass_utils import run_bass_kernel_spmd
from concourse.masks import make_upper_triangular

B, T, C = 4, 2048, 1024
H = 16
D = C // H          # 64
HL = 8              # heads per core
HD = HL * D         # 512: local head dim
N_CORES = 8
CB = C // 128       # 8 c-tiles
TQ_CH = T // 512    # 4 query chunks
TK_TILES = T // 128  # 16 key tiles

F32 = mybir.dt.float32
F32R = mybir.dt.float32r

_compiled = None
TRACE = False          # set True (e.g. from test.py) to neuron-profile the run
LAST_EXEC_NS = None    # filled with max per-core exec_time_ns when TRACE
LAST_TRACE = None      # (insts, trace_path) when TRACE


def _build():
    nc = bacc.Bacc("TRN2", target_bir_lowering=False, debug=False,
                   num_devices=N_CORES)

    xT_ap = nc.dram_tensor("xT", [C, T], F32R, kind="ExternalInput").ap()
    # wqk[n] = 128-column block n of [Wq_shard | Wk_shard], laid out
    # [n, cb, ci, j]: contraction c = cb*128 + ci, output column j.
    wqk_ap = nc.dram_tensor("wqk", [8, CB, 128, 128], F32R, kind="ExternalInput").ap()
    bq_ap = nc.dram_tensor("bq", [4, 128, 1], F32, kind="ExternalInput").ap()
    wv_ap = nc.dram_tensor("wv", [CB, 128, HD], F32R, kind="ExternalInput").ap()
    # wo[g] = rows of Wo for head pair g (head 2g rows 0-63, head 2g+1 rows 64-127)
    wo_ap = nc.dram_tensor("wo", [4, 128, C], F32R, kind="ExternalInput").ap()
    out_ap = nc.dram_tensor("out_p", [T, C], F32, kind="ExternalOutput").ap()

    with tile.TileContext(nc) as tc:
        with (
            tc.tile_pool(name="const", bufs=1) as const_pool,
            tc.tile_pool(name="qkt", bufs=1) as qkt_pool,
            tc.tile_pool(name="v", bufs=1) as v_pool,
        ):
            # Triangular mask (valid = key_i <= query_j).  memset can't emit
            # float32r, so build in f32 and round via a copy.
            trif = const_pool.tile([128, 128], F32)
            make_upper_triangular(nc, trif, val=1.0, diag=True)
            tri = const_pool.tile([128, 128], F32R)
            nc.vector.tensor_copy(tri[:], trif[:])
            onesf = const_pool.tile([128, HL], F32)
            nc.gpsimd.memset(onesf, 1.0)
            bq_t = [const_pool.tile([128, 1], F32, name=f"bq{n}") for n in range(4)]
            for n in range(4):
                nc.sync.dma_start(bq_t[n][:], bq_ap[n])

            QKT = [qkt_pool.tile([128, T], F32R, name=f"qkt{n}") for n in range(8)]
            # V layout [128, 8 heads, 65]: cols 0-63 = head values, col 64 = 1.0
            # (the ones column makes the PV matmul emit the softmax denominator
            # as PSUM row 64 of the same accumulation group).
            V = [v_pool.tile([128, HL, D + 1], F32R, name=f"v{t}")
                 for t in range(TK_TILES)]
            for t in range(TK_TILES):
                nc.vector.tensor_copy(V[t][:, :, D], onesf[:])

            # ---------------- projection phase ----------------
            with (
                tc.tile_pool(name="xt", bufs=1) as xt_pool,
                tc.tile_pool(name="wqk", bufs=2) as wqk_pool,
                tc.tile_pool(name="wv", bufs=1) as wv_pool,
                tc.tile_pool(name="pproj", bufs=4, space="PSUM") as pproj_pool,
            ):
                XT = [xt_pool.tile([128, T], F32R, name=f"xt{cb}") for cb in range(CB)]
                for cb in range(CB):
                    nc.sync.dma_start(XT[cb][:], xT_ap[cb * 128:(cb + 1) * 128, :])

                WV = [wv_pool.tile([128, HD], F32R, name=f"wv{cb}") for cb in range(CB)]
                for cb in range(CB):
                    nc.sync.dma_start(WV[cb][:], wv_ap[cb])

                # QK^T = W^T @ x^T, output rows = qk columns (col block n)
                for n in range(8):
                    wt = wqk_pool.tile([128, CB, 128], F32R)
                    nc.sync.dma_start(wt[:], wqk_ap[n].rearrange("cb ci j -> ci cb j"))
                    for q0 in range(TQ_CH):
                        ps = pproj_pool.tile([128, 512], F32)
                        for cb in range(CB):
                            nc.tensor.matmul(
                                ps[:], wt[:, cb, :],
                                XT[cb][:, q0 * 512:(q0 + 1) * 512],
                                start=(cb == 0), stop=(cb == CB - 1),
                            )
                        dst = QKT[n][:, q0 * 512:(q0 + 1) * 512]
                        if n < 4:
                            nc.vector.tensor_scalar_add(dst, ps[:], bq_t[n][:])
                        else:
                            nc.vector.tensor_copy(dst, ps[:])

                # V = x @ Wv (natural layout)
                for tt in range(TK_TILES):
                    ps = pproj_pool.tile([128, 512], F32, tag="psv")
                    for cb in range(CB):
                        nc.tensor.matmul(
                            ps[:], XT[cb][:, tt * 128:(tt + 1) * 128], WV[cb][:],
                            start=(cb == 0), stop=(cb == CB - 1),
                        )
                    nc.vector.tensor_copy(
                        V[tt][:, :, 0:D],
                        ps[:].rearrange("p (h d) -> p h d", h=HL),
                    )

            # ---------------- attention phase ----------------
            with (
                tc.tile_pool(name="ot", bufs=1) as ot_pool,
                tc.tile_pool(name="wo", bufs=1) as wo_pool,
                tc.tile_pool(name="e", bufs=3) as e_pool,
                tc.tile_pool(name="rb", bufs=2) as rb_pool,
                tc.tile_pool(name="ps_s", bufs=3, space="PSUM") as ps_s_pool,
                tc.tile_pool(name="ps_o", bufs=2, space="PSUM") as ps_o_pool,
                tc.tile_pool(name="ps_p", bufs=2, space="PSUM") as ps_p_pool,
                tc.tile_pool(name="ostg", bufs=3) as ostg_pool,
            ):
                OT = [ot_pool.tile([128, T], F32R, name=f"ot{g}") for g in range(4)]
                WO = [wo_pool.tile([128, C], F32R, name=f"wo{g}") for g in range(4)]
                for g in range(4):
                    nc.sync.dma_start(WO[g][:], wo_ap[g])

                for h in range(HL):
                    po = (h % 2) * 64              # partition offset inside tiles
                    qt = QKT[h // 2]               # q^T rows for heads (2*(h//2), +1)
                    kt = QKT[4 + h // 2]
                    for q0 in range(TQ_CH):
                        tq0 = q0 * 512
                        ntk = q0 * 4 + 4
                        pso = ps_o_pool.tile([65, 512], F32)
                        for tk in range(ntk):
                            r = tk - q0 * 4
                            j0 = r * 128 if r >= 0 else 0
                            pss = ps_s_pool.tile([128, 512], F32)
                            nc.tensor.matmul(
                                pss[:, j0:512],
                                kt[po:po + 64, tk * 128:(tk + 1) * 128],
                                qt[po:po + 64, tq0 + j0:tq0 + 512],
                                start=True, stop=True,
                            )
                            e = e_pool.tile([128, 512], F32R)
                            nc.scalar.activation(
                                e[:, j0:512], pss[:, j0:512],
                                mybir.ActivationFunctionType.Exp,
                                scale=float(D) ** -0.5,
                            )
                            if r >= 0:
                                nc.vector.tensor_mul(
                                    e[:, j0:j0 + 128], e[:, j0:j0 + 128], tri[:]
                                )
                            nc.tensor.matmul(
                                pso[0:65, j0:512],
                                V[tk][:, h, :],
                                e[:, j0:512],
                                start=(tk == 0), stop=(tk == ntk - 1),
                            )
                        recip = rb_pool.tile([1, 512], F32, tag="recip")
                        nc.vector.reciprocal(recip[:], pso[64:65, :])
                        rb = rb_pool.tile([64, 512], F32, tag="rb")
                        nc.gpsimd.partition_broadcast(rb[:], recip[:])
                        nc.vector.tensor_mul(
                            OT[h // 2][po:po + 64, tq0:tq0 + 512],
                            pso[0:64, :], rb[:],
                        )

                # ---------------- output projection ----------------
                for tt in range(TK_TILES):
                    for half in range(2):
                        n0 = half * 512
                        psp = ps_p_pool.tile([128, 512], F32)
                        for g in range(4):
                            nc.tensor.matmul(
                                psp[:],
                                OT[g][:, tt * 128:(tt + 1) * 128],
                                WO[g][:, n0:n0 + 512],
                                start=(g == 0), stop=(g == 3),
                            )
                        ob = ostg_pool.tile([128, 512], F32)
                        nc.vector.tensor_copy(ob[:], psp[:])
                        nc.sync.dma_start(
                            out_ap[tt * 128:(tt + 1) * 128, n0:n0 + 512], ob[:]
                        )

    nc.compile()
    return nc


def _prep_core_inputs(hidden_state, qkv_w, qkv_b, o_w, b, hg):
    """Build the per-core input map for batch b, head group hg."""
    s = slice(hg * HD, (hg + 1) * HD)
    wq = qkv_w[:, 0 * C:1 * C][:, s]          # [C, 512]
    wk = qkv_w[:, 1 * C:2 * C][:, s]          # [C, 512]
    wv = qkv_w[:, 2 * C:3 * C][:, s]          # [C, 512]
    bq = qkv_b[0 * C:1 * C][s]                # [512]

    wqk = np.concatenate([wq, wk], axis=1)    # [C, 1024]
    # [n, cb, ci, j]
    wqk_r = np.ascontiguousarray(
        wqk.reshape(CB, 128, 8, 128).transpose(2, 0, 1, 3)
    )
    bq_r = np.ascontiguousarray(bq.reshape(4, 128, 1))
    wv_r = np.ascontiguousarray(wv.reshape(CB, 128, HD))
    # o_w rows for this head group, regrouped [g, 128, C] in head-pair order
    wo = o_w[hg * HD:(hg + 1) * HD, :]        # [512, C]
    wo_r = np.ascontiguousarray(wo.reshape(4, 128, C))

    xT = np.ascontiguousarray(hidden_state[b].T)  # [C, T]
    return {
        "xT": xT.astype(np.float32),
        "wqk": wqk_r.astype(np.float32),
        "bq": bq_r.astype(np.float32),
        "wv": wv_r.astype(np.float32),
        "wo": wo_r.astype(np.float32),
    }


def _ensure_profile_hook():
    """Register the NTFF profiling hook that this container's antenv lacks.

    The axon boot code registers it via ``antenv.axon_hooks`` when that
    module exists; here we synthesize the module and point it at the same
    ctypes shim over libaxon_pjrt.so.
    """
    import types
    try:
        from antenv.axon_hooks import get_axon_ntff_profile_hook  # noqa: F401
        return
    except ImportError:
        pass
    try:
        import antenv
        from trn_agent_boot.trn_boot import _ntff_profile_via_ctypes
        hook = {"h": _ntff_profile_via_ctypes("/opt/axon/libaxon_pjrt.so")}
        mod = types.ModuleType("antenv.axon_hooks")
        mod.set_axon_ntff_profile_hook = lambda h: hook.__setitem__("h", h)
        mod.get_axon_ntff_profile_hook = lambda: hook["h"]
        sys.modules["antenv.axon_hooks"] = mod
        antenv.axon_hooks = mod
    except Exception as e:  # profiling is best-effort
        print(f"profile hook setup failed: {e}", flush=True)


def kernel(hidden_state, qkv_w, qkv_b, o_w, o_b):
    global _compiled
    hidden_state = np.asarray(hidden_state, dtype=np.float32)
    qkv_w = np.asarray(qkv_w, dtype=np.float32)
    qkv_b = np.asarray(qkv_b, dtype=np.float32)
    o_w = np.asarray(o_w, dtype=np.float32)
    o_b = np.asarray(o_b, dtype=np.float32)

    if _compiled is None:
        _compiled = _build()
    nc = _compiled

    in_maps = []
    for core in range(N_CORES):
        b, hg = core // 2, core % 2
        in_maps.append(_prep_core_inputs(hidden_state, qkv_w, qkv_b, o_w, b, hg))

    global LAST_EXEC_NS, LAST_TRACE
    kw = {}
    if TRACE:
        import tempfile
        _ensure_profile_hook()
        kw = dict(trace=True, tmpdir=tempfile.mkdtemp(prefix="bass_attn_trace_"))
    res = run_bass_kernel_spmd(nc, in_maps, core_ids=list(range(N_CORES)), **kw)
    LAST_EXEC_NS = res.exec_time_ns
    LAST_TRACE = res.instructions_and_trace

    # host-side gather: sum the two head-group partials per batch and add the
    # affine correction (v-bias pushed through Wo, plus o-bias).
    bv = qkv_b[2 * C:3 * C]                   # [C]
    corr = (bv @ o_w + o_b).astype(np.float32)
    out = np.empty((B, T, C), dtype=np.float32)
    for b in range(B):
        p0 = res.results[2 * b]["out_p"]
        p1 = res.results[2 * b + 1]["out_p"]
        out[b] = p0 + p1 + corr
    return out


# revision 22
# speedup vs baseline: 1.7326x; 1.7326x over previous
r"""Causal multi-head attention (B=4, T=2048, C=1024, H=16, D=64) on 8 TRN2 NeuronCores.

Sharding: core = (batch b, head-group hg).  b = core // 2, hg = core % 2.
Each core computes, for its batch, the attention-output contribution of its
8 heads, including the qkv projection restricted to those heads' columns and
the o-projection restricted to those heads' rows.  The two cores sharing a
batch produce partial sums of the o-projection; the host adds them together
with the (analytically folded) v-bias/o-bias correction.

Math notes:
  - k-bias contributes only q-row-constant score shifts, which cancel in
    softmax, so it is dropped; only the q bias is applied on device.
  - v bias and o bias are affine post-softmax:  (P@(V + 1 b_v^T))@Wo + b_o =
    (P@V)@Wo + (b_v@Wo + b_o), folded into a host-side correction row.
  - Softmax is computed without max subtraction (scores are O(1) here), as
    exp(s/8) accumulated straight into the PV matmul; the denominator rides
    as a 65th output row via a ones column interleaved into the V layout.

On-device dataflow per core (matmuls in float32r: full-speed fp32 path):
  xT [C=1024, T=2048] (host pre-transposed) -> SBUF
  QK^T = Wqk^T @ xT  -> [1024, 2048]  (q rows 0-511 with bias, k rows 512-1023)
  V    = xT^T @ Wv   -> [2048, 8, 65] (natural layout + ones column)
  per head h, per 512-wide q chunk:
     S^T tile [tk=128, tq<=512] = K_h @ Q_h^T   (causal-skipped tiles omitted)
     E = exp(S^T / 8)   (diagonal tiles masked with a triangular 0/1 mask)
     [O^T; denom] += [V_h | 1]^T @ E    (one PSUM accumulation group, M=65)
     O^T_norm = O^T * broadcast(1/denom)
  out_partial [2048, 1024] = concat_h(O^T_norm)^T @ Wo_shard  -> DRAM
"""

import sys

sys.path.insert(0, "/opt/trn_rl_repo")

import numpy as np

import concourse.bass as bass
import concourse.tile as tile
from concourse import bacc, mybir
from concourse.bass_utils import run_bass_kernel_spmd
from concourse.masks import make_upper_triangular

B, T, C = 4, 2048, 1024
H = 16
D = C // H          # 64
HL = 8              # heads per core
HD = HL * D         # 512: local head dim
N_CORES = 8
CB = C // 128       # 8 c-tiles
TQ_CH = T // 512    # 4 query chunks
TK_TILES = T // 128  # 16 key tiles

F32 = mybir.dt.float32
F32R = mybir.dt.float32r
BF16 = mybir.dt.bfloat16

_compiled = None
TRACE = False          # set True (e.g. from test.py) to neuron-profile the run
LAST_EXEC_NS = None    # filled with max per-core exec_time_ns when TRACE
LAST_TRACE = None      # (insts, trace_path) when TRACE


def _build():
    nc = bacc.Bacc("TRN2", target_bir_lowering=False, debug=False,
                   num_devices=N_CORES)

    xT_ap = nc.dram_tensor("xT", [C, T], BF16, kind="ExternalInput").ap()
    # wqk[n] = 128-column block n of [Wq_shard | Wk_shard], laid out
    # [n, cb, ci, j]: contraction c = cb*128 + ci, output column j.
    wqk_ap = nc.dram_tensor("wqk", [8, CB, 128, 128], BF16, kind="ExternalInput").ap()
    bq_ap = nc.dram_tensor("bq", [4, 128, 1], F32, kind="ExternalInput").ap()
    wv_ap = nc.dram_tensor("wv", [CB, 128, HD], BF16, kind="ExternalInput").ap()
    # wo[g] = rows of Wo for head pair g (head 2g rows 0-63, head 2g+1 rows 64-127)
    wo_ap = nc.dram_tensor("wo", [4, 128, C], BF16, kind="ExternalInput").ap()
    out_ap = nc.dram_tensor("out_p", [T, C], F32, kind="ExternalOutput").ap()

    with tile.TileContext(nc) as tc:
        with (
            tc.tile_pool(name="const", bufs=1) as const_pool,
            tc.tile_pool(name="qkt", bufs=1) as qkt_pool,
            tc.tile_pool(name="v", bufs=1) as v_pool,
        ):
            # Triangular mask (valid = key_i <= query_j).  memset can't emit
            # float32r, so build in f32 and round via a copy.
            trif = const_pool.tile([128, 128], F32)
            make_upper_triangular(nc, trif, val=1.0, diag=True)
            tri = const_pool.tile([128, 128], BF16)
            nc.vector.tensor_copy(tri[:], trif[:])
            onesf = const_pool.tile([128, HL], F32)
            nc.gpsimd.memset(onesf, 1.0)
            bq_t = [const_pool.tile([128, 1], F32, name=f"bq{n}") for n in range(4)]
            for n in range(4):
                nc.sync.dma_start(bq_t[n][:], bq_ap[n])

            QKT = [qkt_pool.tile([128, T], BF16, name=f"qkt{n}") for n in range(8)]
            # V layout [128, 8 heads, 65]: cols 0-63 = head values, col 64 = 1.0
            # (the ones column makes the PV matmul emit the softmax denominator
            # as PSUM row 64 of the same accumulation group).
            V = [v_pool.tile([128, HL, D + 1], BF16, name=f"v{t}")
                 for t in range(TK_TILES)]
            for t in range(TK_TILES):
                nc.vector.tensor_copy(V[t][:, :, D], onesf[:])

            # ---------------- projection phase ----------------
            with (
                tc.tile_pool(name="xt", bufs=1) as xt_pool,
                tc.tile_pool(name="wqk", bufs=2) as wqk_pool,
                tc.tile_pool(name="wv", bufs=1) as wv_pool,
                tc.tile_pool(name="pproj", bufs=4, space="PSUM") as pproj_pool,
            ):
                XT = [xt_pool.tile([128, T], BF16, name=f"xt{cb}") for cb in range(CB)]
                for cb in range(CB):
                    nc.sync.dma_start(XT[cb][:], xT_ap[cb * 128:(cb + 1) * 128, :])

                WV = [wv_pool.tile([128, HD], BF16, name=f"wv{cb}") for cb in range(CB)]
                for cb in range(CB):
                    nc.sync.dma_start(WV[cb][:], wv_ap[cb])

                # QK^T = W^T @ x^T, output rows = qk columns (col block n)
                for n in range(8):
                    wt = wqk_pool.tile([128, CB, 128], BF16)
                    nc.sync.dma_start(wt[:], wqk_ap[n].rearrange("cb ci j -> ci cb j"))
                    for q0 in range(TQ_CH):
                        ps = pproj_pool.tile([128, 512], F32)
                        for cb in range(CB):
                            nc.tensor.matmul(
                                ps[:], wt[:, cb, :],
                                XT[cb][:, q0 * 512:(q0 + 1) * 512],
                                start=(cb == 0), stop=(cb == CB - 1),
                            )
                        dst = QKT[n][:, q0 * 512:(q0 + 1) * 512]
                        if n < 4:
                            nc.vector.tensor_scalar_add(dst, ps[:], bq_t[n][:])
                        else:
                            nc.vector.tensor_copy(dst, ps[:])

                # V = x @ Wv (natural layout)
                for tt in range(TK_TILES):
                    ps = pproj_pool.tile([128, 512], F32, tag="psv")
                    for cb in range(CB):
                        nc.tensor.matmul(
                            ps[:], XT[cb][:, tt * 128:(tt + 1) * 128], WV[cb][:],
                            start=(cb == 0), stop=(cb == CB - 1),
                        )
                    nc.vector.tensor_copy(
                        V[tt][:, :, 0:D],
                        ps[:].rearrange("p (h d) -> p h d", h=HL),
                    )

            # ---------------- attention phase ----------------
            with (
                tc.tile_pool(name="ot", bufs=1) as ot_pool,
                tc.tile_pool(name="wo", bufs=1) as wo_pool,
                tc.tile_pool(name="e", bufs=3) as e_pool,
                tc.tile_pool(name="rb", bufs=2) as rb_pool,
                tc.tile_pool(name="ps_s", bufs=3, space="PSUM") as ps_s_pool,
                tc.tile_pool(name="ps_o", bufs=2, space="PSUM") as ps_o_pool,
                tc.tile_pool(name="ps_p", bufs=2, space="PSUM") as ps_p_pool,
                tc.tile_pool(name="ostg", bufs=3) as ostg_pool,
                tc.tile_pool(name="rd", bufs=2, space="DRAM") as rd_pool,
            ):
                OT = [ot_pool.tile([128, T], BF16, name=f"ot{g}") for g in range(4)]
                WO = [wo_pool.tile([128, C], BF16, name=f"wo{g}") for g in range(4)]
                for g in range(4):
                    nc.sync.dma_start(WO[g][:], wo_ap[g])

                for q0 in range(TQ_CH):
                    tq0 = q0 * 512
                    ntk = q0 * 4 + 4
                    den_d = rd_pool.tile([HL, 512], F32, tag="dend")
                    for h in range(HL):
                        po = (h % 2) * 64          # partition offset inside tiles
                        qt = QKT[h // 2]           # q^T rows for heads (2*(h//2), +1)
                        kt = QKT[4 + h // 2]
                        pso = ps_o_pool.tile([65, 512], F32)
                        for tk in range(ntk):
                            r = tk - q0 * 4
                            j0 = r * 128 if r >= 0 else 0
                            pss = ps_s_pool.tile([128, 512], F32)
                            nc.tensor.matmul(
                                pss[:, j0:512],
                                kt[po:po + 64, tk * 128:(tk + 1) * 128],
                                qt[po:po + 64, tq0 + j0:tq0 + 512],
                                start=True, stop=True,
                            )
                            e = e_pool.tile([128, 512], BF16)
                            nc.scalar.activation(
                                e[:, j0:512], pss[:, j0:512],
                                mybir.ActivationFunctionType.Exp,
                                scale=float(D) ** -0.5,
                            )
                            if r >= 0:
                                nc.vector.tensor_mul(
                                    e[:, j0:j0 + 128], e[:, j0:j0 + 128], tri[:]
                                )
                            nc.tensor.matmul(
                                pso[0:65, j0:512],
                                V[tk][:, h, :],
                                e[:, j0:512],
                                start=(tk == 0), stop=(tk == ntk - 1),
                            )
                        # evacuate PSUM: unnormalized O^T and the denominator row
                        nc.vector.tensor_copy(
                            OT[h // 2][po:po + 64, tq0:tq0 + 512], pso[0:64, :]
                        )
                        sden = rb_pool.tile([1, 512], F32, tag="sden")
                        nc.vector.tensor_copy(sden[:], pso[64:65, :])
                        nc.sync.dma_start(den_d[h:h + 1, :], sden[:])
                    # one exact reciprocal for all 8 heads of this chunk,
                    # then broadcast to 64 partitions via a DRAM bounce
                    den = rb_pool.tile([HL, 512], F32, tag="den")
                    nc.sync.dma_start(den[:], den_d[:])
                    recf = rb_pool.tile([HL, 512], F32, tag="recf")
                    nc.vector.reciprocal(recf[:], den[:])
                    rec = rb_pool.tile([HL, 512], BF16, tag="rec")
                    nc.vector.tensor_copy(rec[:], recf[:])
                    rec_d = rd_pool.tile([HL, 512], BF16)
                    nc.sync.dma_start(rec_d[:], rec[:])
                    rb_all = rb_pool.tile([128, HL, 512], BF16, tag="rball")
                    nc.sync.dma_start(
                        rb_all[:], rec_d[:][None].broadcast_to([128, HL, 512])
                    )
                    for h in range(HL):
                        po = (h % 2) * 64
                        dst = OT[h // 2][po:po + 64, tq0:tq0 + 512]
                        nc.vector.tensor_mul(dst, dst, rb_all[po:po + 64, h, :])

                # ---------------- output projection ----------------
                for tt in range(TK_TILES):
                    for half in range(2):
                        n0 = half * 512
                        psp = ps_p_pool.tile([128, 512], F32)
                        for g in range(4):
                            nc.tensor.matmul(
                                psp[:],
                                OT[g][:, tt * 128:(tt + 1) * 128],
                                WO[g][:, n0:n0 + 512],
                                start=(g == 0), stop=(g == 3),
                            )
                        ob = ostg_pool.tile([128, 512], F32)
                        nc.vector.tensor_copy(ob[:], psp[:])
                        nc.sync.dma_start(
                            out_ap[tt * 128:(tt + 1) * 128, n0:n0 + 512], ob[:]
                        )

    nc.compile()
    return nc


def _prep_core_inputs(hidden_state, qkv_w, qkv_b, o_w, b, hg):
    """Build the per-core input map for batch b, head group hg."""
    s = slice(hg * HD, (hg + 1) * HD)
    wq = qkv_w[:, 0 * C:1 * C][:, s]          # [C, 512]
    wk = qkv_w[:, 1 * C:2 * C][:, s]          # [C, 512]
    wv = qkv_w[:, 2 * C:3 * C][:, s]          # [C, 512]
    bq = qkv_b[0 * C:1 * C][s]                # [512]

    wqk = np.concatenate([wq, wk], axis=1)    # [C, 1024]
    # [n, cb, ci, j]
    wqk_r = np.ascontiguousarray(
        wqk.reshape(CB, 128, 8, 128).transpose(2, 0, 1, 3)
    )
    bq_r = np.ascontiguousarray(bq.reshape(4, 128, 1))
    wv_r = np.ascontiguousarray(wv.reshape(CB, 128, HD))
    # o_w rows for this head group, regrouped [g, 128, C] in head-pair order
    wo = o_w[hg * HD:(hg + 1) * HD, :]        # [512, C]
    wo_r = np.ascontiguousarray(wo.reshape(4, 128, C))

    import ml_dtypes
    bf16 = ml_dtypes.bfloat16
    xT = np.ascontiguousarray(hidden_state[b].T)  # [C, T]
    return {
        "xT": xT.astype(bf16),
        "wqk": wqk_r.astype(bf16),
        "bq": bq_r.astype(np.float32),
        "wv": wv_r.astype(bf16),
        "wo": wo_r.astype(bf16),
    }


def _ensure_profile_hook():
    """Register the NTFF profiling hook that this container's antenv lacks.

    The axon boot code registers it via ``antenv.axon_hooks`` when that
    module exists; here we synthesize the module and point it at the same
    ctypes shim over libaxon_pjrt.so.
    """
    import types
    try:
        from antenv.axon_hooks import get_axon_ntff_profile_hook  # noqa: F401
        return
    except ImportError:
        pass
    try:
        import antenv
        from trn_agent_boot.trn_boot import _ntff_profile_via_ctypes
        hook = {"h": _ntff_profile_via_ctypes("/opt/axon/libaxon_pjrt.so")}
        mod = types.ModuleType("antenv.axon_hooks")
        mod.set_axon_ntff_profile_hook = lambda h: hook.__setitem__("h", h)
        mod.get_axon_ntff_profile_hook = lambda: hook["h"]
        sys.modules["antenv.axon_hooks"] = mod
        antenv.axon_hooks = mod
    except Exception as e:  # profiling is best-effort
        print(f"profile hook setup failed: {e}", flush=True)


def kernel(hidden_state, qkv_w, qkv_b, o_w, o_b):
    global _compiled
    hidden_state = np.asarray(hidden_state, dtype=np.float32)
    qkv_w = np.asarray(qkv_w, dtype=np.float32)
    qkv_b = np.asarray(qkv_b, dtype=np.float32)
    o_w = np.asarray(o_w, dtype=np.float32)
    o_b = np.asarray(o_b, dtype=np.float32)

    if _compiled is None:
        _compiled = _build()
    nc = _compiled

    in_maps = []
    for core in range(N_CORES):
        b, hg = core // 2, core % 2
        in_maps.append(_prep_core_inputs(hidden_state, qkv_w, qkv_b, o_w, b, hg))

    global LAST_EXEC_NS, LAST_TRACE
    kw = {}
    if TRACE:
        import tempfile
        _ensure_profile_hook()
        kw = dict(trace=True, tmpdir=tempfile.mkdtemp(prefix="bass_attn_trace_"))
    res = run_bass_kernel_spmd(nc, in_maps, core_ids=list(range(N_CORES)), **kw)
    LAST_EXEC_NS = res.exec_time_ns
    LAST_TRACE = res.instructions_and_trace

    # host-side gather: sum the two head-group partials per batch and add the
    # affine correction (v-bias pushed through Wo, plus o-bias).
    bv = qkv_b[2 * C:3 * C]                   # [C]
    corr = (bv @ o_w + o_b).astype(np.float32)
    out = np.empty((B, T, C), dtype=np.float32)
    for b in range(B):
        p0 = res.results[2 * b]["out_p"]
        p1 = res.results[2 * b + 1]["out_p"]
        out[b] = p0 + p1 + corr
    return out
